# revision 6
# baseline (speedup 1.0000x reference)
"""Trainium2 Bass kernel for nn_Encoder (GNN: BN+GAT+BN+GCN+BN+GCN+VAE head).

Self-contained: takes full inputs (as in reference.setup_inputs()), shards
destination nodes across 8 NeuronCores, runs one SPMD Bass program, returns
(z_mean, z_log_var, z).

Per core (dst-shard of N/8 nodes; its in-edges are a contiguous dst-sorted
range):
  Phase A: h1 = x @ W1fold + c1 (feature-major matmuls), att = h1 @ [a_i|a_j];
           write per-node 192-float rows [h1 | att_j | pad] to a table;
           AllGather the table.
  GAT: per 128-node dst block, dma_gather the 768B source rows of its edges
       (int16 indices -> 4 subtables of 25000 rows; edge chunks of 128 are
       (block, subtable)-pure, schedule padded to the max over cores so the
       SPMD program is identical on every core). One-hot(dst)*p matrices are
       built on DVE in one fused tensor_scalar; PE matmuls scatter-accumulate
       [sum p*h | sum p] into PSUM. att_i is broadcast per block via a rank-1
       PE matmul and reduced per edge with an ACT accumulate. Softmax uses no
       max-subtraction (score range is fp32-safe). Block drain computes out1
       and immediately h2 = out1@W2fold + c2 -> table 2 (so the next pass
       gathers 64-wide rows).
  GCN2/GCN3: same skeleton with one-hot(dst)*edge_w; GCN3's drain runs the
       VAE head inline. All BN layers are folded into weights on the host.
"""

import math
import numpy as np

NCORES = 8
BN_EPS = 1e-3
BLK = 128         # dst-block / one-hot window (nodes)
CHUNK = 128       # edges per scatter matmul (PE contraction dim)
NST = 4           # gather subtables: A0,A1 (first halves), B0,B1 (second)
STHALF = 32768    # int16-positive row limit per subtable
SPB = 2           # blocks per gather span
MAXC = 8          # max chunks (x128 idxs) per dma_gather call (desc-ring safe)

f32 = np.float32


# ----------------------------------------------------------------- host prep

def _fold_weights(i):
    s1 = (i["bn1_gamma"] / np.sqrt(i["bn1_var"] + BN_EPS)).astype(f32)
    t1 = (i["bn1_beta"] - i["bn1_mean"] * s1).astype(f32)
    W1 = (s1[:, None] * i["gat_w"]).astype(f32)            # [128,128]
    c1 = (t1 @ i["gat_w"]).astype(f32)                     # [128]
    s2 = (i["bn2_gamma"] / np.sqrt(i["bn2_var"] + BN_EPS)).astype(f32)
    t2 = (i["bn2_beta"] - i["bn2_mean"] * s2).astype(f32)
    W2 = (s2[:, None] * i["gcn2_w"]).astype(f32)           # [128,64]
    c2 = (t2 @ i["gcn2_w"]).astype(f32)                    # [64]
    s3 = (i["bn3_gamma"] / np.sqrt(i["bn3_var"] + BN_EPS)).astype(f32)
    t3 = (i["bn3_beta"] - i["bn3_mean"] * s3).astype(f32)
    W3 = (s3[:, None] * i["gcn3_w"]).astype(f32)           # [64,32]
    c3 = (t3 @ i["gcn3_w"]).astype(f32)                    # [32]
    B = BLK
    return dict(
        W1=W1, c1=c1[:, None].copy(),                       # c1 as [128,1]
        aa=np.stack([i["gat_a_self"], i["gat_a_nbr"]], 1).astype(f32),  # [128,2]
        W2=W2, c2b=np.tile(c2, (B, 1)),                     # [128,64]
        W3=W3, c3b=np.tile(c3, (B, 1)),                     # [128,32]
        b1b=np.tile(i["gat_b"].astype(f32), (B, 1)),        # [128,128]
        b2b=np.tile(i["gcn2_b"].astype(f32), (B, 1)),       # [128,64]
        b3b=np.tile(i["gcn3_b"].astype(f32), (B, 1)),       # [128,32]
        zmzvw=np.concatenate([i["zm_w"], i["zv_w"]], 1).astype(f32),  # [32,128]
        zmzvb=np.tile(np.concatenate([i["zm_b"], i["zv_b"]]).astype(f32),
                      (B, 1)),                              # [128,128]
    )


class Sched:
    pass


def _edge_schedule(edge_src, edge_dst, edge_w, N):
    """Static SPMD schedule. Chunk order: span -> subtable -> block.
    Per-(block,subtable) chunk counts are the max over cores."""
    SH = N // NCORES
    NBLK = math.ceil(SH / BLK)
    H = (NBLK // 2) * BLK                 # per-core first-half node count
    NA = NCORES * H
    NB = N - NA
    assert NA <= 2 * STHALF and NB <= 2 * STHALF

    def st_loc(src):
        c = src // SH
        r = src - c * SH
        inA = r < H
        row = np.where(inA, c * H + r, c * (SH - H) + (r - H))
        st = np.where(inA, (row >= STHALF).astype(np.int64),
                      2 + (row >= STHALF).astype(np.int64))
        loc = row - (st % 2) * STHALF
        return st, loc

    bounds = np.searchsorted(edge_dst, np.arange(NCORES + 1) * SH)
    cnts = np.zeros((NCORES, NBLK, NST), np.int64)
    for c in range(NCORES):
        e = slice(bounds[c], bounds[c + 1])
        b = (edge_dst[e] - c * SH) // BLK
        st, _ = st_loc(edge_src[e])
        np.add.at(cnts[c], (b, st), 1)
    CPBS = -(-cnts.max(0) // CHUNK)                        # [NBLK, NST]
    CPBS[:, 0] = np.maximum(1, CPBS[:, 0])                 # every block runs
    spans = [(b0, min(b0 + SPB, NBLK)) for b0 in range(0, NBLK, SPB)]

    # global chunk bases in span->st->block order
    c0 = np.zeros((NBLK, NST), np.int64)
    acc = 0
    for (b0, b1) in spans:
        for st in range(NST):
            for b in range(b0, b1):
                c0[b, st] = acc
                acc += CPBS[b, st]
    TOTC = acc

    # per-span metadata
    span_meta = []
    for (b0, b1) in spans:
        sc0 = int(c0[b0, 0])                               # first chunk of span
        nch = int(CPBS[b0:b1].sum())
        calls = []
        for st in range(NST):
            r0 = int(c0[b0, st])
            rn = int(CPBS[b0:b1, st].sum())
            while rn > 0:
                n = min(rn, MAXC)
                calls.append((st, r0 - sc0, n))            # (subtable, slot0, nch)
                r0 += n
                rn -= n
        blocks = []
        for b in range(b0, b1):
            runs = [(st, int(c0[b, st]) - sc0, int(CPBS[b, st]))
                    for st in range(NST) if CPBS[b, st] > 0]
            blocks.append((b, runs))
        span_meta.append((sc0, nch, calls, blocks))
    SMAX = max(m[1] for m in span_meta)
    CPBMAX = int(CPBS.sum(1).max())

    # streams
    gidx = np.zeros((NCORES, 16, TOTC * 8), np.int16)
    dloc = np.full((NCORES, 128, TOTC), -1.0, f32)
    wstr = np.zeros((NCORES, 128, TOTC), f32)
    for c in range(NCORES):
        e = slice(bounds[c], bounds[c + 1])
        src = edge_src[e]
        d = edge_dst[e] - c * SH
        w = edge_w[e]
        b = d // BLK
        st, loc = st_loc(src)
        key = b * NST + st
        order = np.argsort(key, kind="stable")
        src, d, w, b, st, key, loc = (a[order]
                                      for a in (src, d, w, b, st, key, loc))
        kcnt = np.bincount(key, minlength=NBLK * NST)
        kstart = np.concatenate([[0], np.cumsum(kcnt)])[:-1]
        pos = np.arange(len(src)) - kstart[key]
        col = c0[b, st] + pos // CHUNK
        row = pos % CHUNK
        gidx[c, row % 16, col * 8 + row // 16] = loc.astype(np.int16)
        dloc[c, row, col] = (d - b * BLK).astype(f32)
        wstr[c, row, col] = w

    s = Sched()
    s.SH, s.NBLK, s.TOTC, s.SMAX, s.CPBMAX = SH, NBLK, TOTC, SMAX, CPBMAX
    s.CPBS, s.c0, s.span_meta = CPBS, c0, span_meta
    s.LASTB = SH - BLK * (NBLK - 1)
    s.H, s.NA, s.NB = H, NA, NB
    # subtable -> (half, base, size)
    s.stdef = [(0, 0, min(STHALF, NA)), (0, STHALF, max(0, NA - STHALF)),
               (1, 0, min(STHALF, NB)), (1, STHALF, max(0, NB - STHALF))]
    return s, np.tile(gidx, (1, 8, 1)), dloc, wstr


# ------------------------------------------------------------- device program

def _build_program(N, s):
    import concourse.tile as tile
    from concourse import bacc, mybir
    from concourse.masks import make_identity
    import contextlib

    dt = mybir.dt
    AF = mybir.ActivationFunctionType
    OP = mybir.AluOpType
    SH, NBLK, TOTC, SMAX = s.SH, s.NBLK, s.TOTC, s.SMAX
    LASTB = s.LASTB

    nc = bacc.Bacc("TRN2", num_devices=NCORES, target_bir_lowering=False,
                   dynamic_dma_scratch_size=16384)

    xT_in = nc.dram_tensor("xT", [128, SH], dt.float32, kind="ExternalInput")
    eps_in = nc.dram_tensor("eps_s", [SH, 64], dt.float32, kind="ExternalInput")
    gidx_in = nc.dram_tensor("gidx", [128, TOTC * 8], dt.int16, kind="ExternalInput")
    dloc_in = nc.dram_tensor("dloc", [128, TOTC], dt.float32, kind="ExternalInput")
    wstr_in = nc.dram_tensor("wstr", [128, TOTC], dt.float32, kind="ExternalInput")
    wshapes = dict(W1=[128, 128], c1=[128, 1], aa=[128, 2], W2=[128, 64],
                   c2b=[BLK, 64], W3=[64, 32], c3b=[BLK, 32], b1b=[BLK, 128],
                   b2b=[BLK, 64], b3b=[BLK, 32], zmzvw=[32, 128],
                   zmzvb=[BLK, 128])
    win = {n: nc.dram_tensor(n, sh, dt.float32, kind="ExternalInput")
           for n, sh in wshapes.items()}
    iota_in = nc.dram_tensor("iota", [128, BLK], dt.float32, kind="ExternalInput")
    ones_in = nc.dram_tensor("ones_row", [1, 128], dt.float32, kind="ExternalInput")

    zm_o = nc.dram_tensor("zm_o", [SH, 64], dt.float32, kind="ExternalOutput")
    zlv_o = nc.dram_tensor("zlv_o", [SH, 64], dt.float32, kind="ExternalOutput")
    z_o = nc.dram_tensor("z_o", [SH, 64], dt.float32, kind="ExternalOutput")

    H, NA, NB = s.H, s.NA, s.NB
    aug_sh = nc.dram_tensor("aug_sh", [SH, 192], dt.float32)
    T1A = nc.dram_tensor("T1A", [NA, 192], dt.float32)
    T1B = nc.dram_tensor("T1B", [NB, 192], dt.float32)
    t2_sh = nc.dram_tensor("t2_sh", [SH, 64], dt.float32)
    T2A = nc.dram_tensor("T2A", [NA, 64], dt.float32)
    T2B = nc.dram_tensor("T2B", [NB, 64], dt.float32)
    t3_sh = nc.dram_tensor("t3_sh", [SH, 64], dt.float32)
    T3A = nc.dram_tensor("T3A", [NA, 64], dt.float32)
    T3B = nc.dram_tensor("T3B", [NB, 64], dt.float32)
    atti_d = nc.dram_tensor("atti_d", [1, SH], dt.float32)

    RG = [list(range(NCORES))]

    def st_ap(tabs, st):
        half, base, size = s.stdef[st]
        return tabs[half][base:base + size, :]

    with tile.TileContext(nc, num_cores=NCORES) as tc:
        with contextlib.ExitStack() as ctx:
            from concourse import library_config
            nc.gpsimd.load_library(library_config.mlp)
            pers = ctx.enter_context(tc.tile_pool(name="pers", bufs=1))
            W = {}
            for n, sh in wshapes.items():
                W[n] = pers.tile(sh, dt.float32, name=f"w_{n}")
                nc.sync.dma_start(out=W[n][:], in_=win[n][:])
            iotas = pers.tile([128, BLK], dt.float32)
            nc.sync.dma_start(out=iotas[:], in_=iota_in[:])
            oness = pers.tile([1, 128], dt.float32)
            nc.sync.dma_start(out=oness[:], in_=ones_in[:])
            idents = pers.tile([128, 128], dt.float32)
            make_identity(nc, idents[:])
            gidxs = pers.tile([128, TOTC * 8], dt.int16)
            nc.sync.dma_start(out=gidxs[:], in_=gidx_in[:])
            dlocs = pers.tile([128, TOTC], dt.float32)
            nc.sync.dma_start(out=dlocs[:], in_=dloc_in[:])
            wstrs = pers.tile([128, TOTC], dt.float32)
            nc.sync.dma_start(out=wstrs[:], in_=wstr_in[:])

            # ---------------- phase A ---------------------------------------
            with tc.tile_pool(name="pa_sb", bufs=3) as pa, \
                 tc.tile_pool(name="pa_xt", bufs=1) as paxt, \
                 tc.tile_pool(name="pa_ps", bufs=2, space="PSUM") as pap, \
                 tc.tile_pool(name="pa_ps2", bufs=2, space="PSUM") as pap2:
                xTs = paxt.tile([128, SH], dt.float32)
                nc.sync.dma_start(out=xTs[:], in_=xT_in[:])
                for sb_i in range(NBLK):
                    n0 = sb_i * BLK
                    ns = min(BLK, SH - n0)
                    h1T_ps = pap.tile([128, BLK], dt.float32, tag="h1T")
                    nc.tensor.matmul(out=h1T_ps[:, :ns], lhsT=W["W1"][:],
                                     rhs=xTs[:, n0:n0 + ns], start=True, stop=True)
                    h1T_sb = pa.tile([128, BLK], dt.float32, tag="h1Ts")
                    nc.scalar.activation(out=h1T_sb[:, :ns], in_=h1T_ps[:, :ns],
                                         func=AF.Identity, bias=W["c1"][:])
                    att_ps = pap2.tile([2, BLK], dt.float32, tag="att")
                    nc.tensor.matmul(out=att_ps[:, :ns], lhsT=W["aa"][:],
                                     rhs=h1T_sb[:, :ns], start=True, stop=True)
                    att_sb = pa.tile([2, BLK], dt.float32, tag="att_sb")
                    nc.vector.tensor_copy(out=att_sb[:, :ns], in_=att_ps[:, :ns])
                    nc.sync.dma_start(out=atti_d[0:1, n0:n0 + ns],
                                      in_=att_sb[0:1, :ns])
                    tr_ps = pap.tile([128, BLK], dt.float32, tag="tr")
                    nc.tensor.transpose(out=tr_ps[:ns, :], in_=h1T_sb[:, :ns],
                                        identity=idents[:])
                    attT_ps = pap2.tile([128, 2], dt.float32, tag="attT")
                    nc.tensor.transpose(out=attT_ps[:ns, :], in_=att_sb[:, :ns],
                                        identity=idents[0:2, 0:2])
                    aug_sb = pa.tile([128, 192], dt.float32, tag="aug")
                    nc.vector.tensor_copy(out=aug_sb[:ns, 0:128], in_=tr_ps[:ns, :])
                    nc.vector.tensor_copy(out=aug_sb[:ns, 128:129],
                                          in_=attT_ps[:ns, 1:2])
                    nc.vector.memset(aug_sb[:ns, 129:192], 0.0)
                    nc.sync.dma_start(out=aug_sh[n0:n0 + ns, :], in_=aug_sb[:ns, :])

            nc.gpsimd.collective_compute(
                "AllGather", mybir.AluOpType.bypass, replica_groups=RG,
                ins=[aug_sh[0:H, :]], outs=[T1A[:]])
            nc.gpsimd.collective_compute(
                "AllGather", mybir.AluOpType.bypass, replica_groups=RG,
                ins=[aug_sh[H:SH, :]], outs=[T1B[:]])

            # ---------------- GAT pass --------------------------------------
            with tc.tile_pool(name="g1", bufs=2) as g1pool, \
                 tc.tile_pool(name="gsb", bufs=2) as gsb, \
                 tc.tile_pool(name="gsc", bufs=2) as gsc, \
                 tc.tile_pool(name="pacc", bufs=2, space="PSUM") as pacc, \
                 tc.tile_pool(name="pab", bufs=2, space="PSUM") as pabp, \
                 tc.tile_pool(name="ptr", bufs=2, space="PSUM") as ptrp, \
                 tc.tile_pool(name="pmm", bufs=2, space="PSUM") as pmmp:
                for (sc0, nch, calls, blocks) in s.span_meta:
                    gbuf = g1pool.tile([128, SMAX, 192], dt.float32, tag="g1b")
                    for (st, k0, n) in calls:
                        nc.gpsimd.dma_gather(
                            out_ap=gbuf[:, k0:k0 + n, :],
                            in_ap=st_ap((T1A, T1B), st),
                            idxs_ap=gidxs[:, (sc0 + k0) * 8:(sc0 + k0 + n) * 8],
                            num_idxs=n * CHUNK, num_idxs_reg=n * CHUNK,
                            elem_size=192)
                    for (b, runs) in blocks:
                        bn = LASTB if b == NBLK - 1 else BLK
                        # att_i broadcast for this block
                        atti_sb = gsb.tile([1, BLK], dt.float32, tag="atti")
                        nc.sync.dma_start(out=atti_sb[0:1, :bn],
                                          in_=atti_d[0:1, b * BLK:b * BLK + bn])
                        if bn < BLK:
                            nc.vector.memset(atti_sb[0:1, bn:BLK], 0.0)
                        ab_ps = pabp.tile([128, BLK], dt.float32, tag="ab")
                        nc.tensor.matmul(out=ab_ps[:], lhsT=oness[:],
                                         rhs=atti_sb[:], start=True, stop=True)
                        ab_sb = gsb.tile([128, BLK], dt.float32, tag="ab_sb")
                        nc.vector.tensor_copy(out=ab_sb[:], in_=ab_ps[:])
                        attie = gsc.tile([128, s.CPBMAX], dt.float32, tag="attie")
                        vb = gsc.tile([128, s.CPBMAX], dt.float32, tag="vb")
                        v2 = gsc.tile([128, s.CPBMAX], dt.float32, tag="v2")
                        pb = gsc.tile([128, s.CPBMAX], dt.float32, tag="pb")
                        ohsc = gsc.tile([128, BLK], dt.float32, tag="ohsc")
                        acc_ps = pacc.tile([128, 129], dt.float32, tag="acc")
                        nch_b = sum(n for (_, _, n) in runs)
                        done = 0
                        j = 0
                        for (st, k0, n) in runs:
                            for i in range(n):
                                ch = sc0 + k0 + i
                                oh = gsb.tile([128, BLK], dt.float32, tag="oh")
                                nc.vector.tensor_scalar(
                                    out=oh[:], in0=iotas[:],
                                    scalar1=dlocs[:, ch:ch + 1], scalar2=None,
                                    op0=OP.is_equal)
                                ohA = gsb.tile([128, BLK], dt.float32, tag="ohA")
                                nc.gpsimd.tensor_tensor(out=ohA[:], in0=oh[:],
                                                        in1=ab_sb[:], op=OP.mult)
                                nc.scalar.activation(out=ohsc[:], in_=ohA[:],
                                                     func=AF.Identity,
                                                     accum_out=attie[:, j + i:j + i + 1])
                            # batched v = attj + attie ; p = exp(max(v, .2v))
                            nc.vector.tensor_tensor(
                                out=vb[:, j:j + n], in0=gbuf[:, k0:k0 + n, 128],
                                in1=attie[:, j:j + n], op=OP.add)
                            nc.vector.tensor_scalar_mul(out=v2[:, j:j + n],
                                                        in0=vb[:, j:j + n],
                                                        scalar1=0.2)
                            nc.vector.tensor_tensor(out=vb[:, j:j + n],
                                                    in0=vb[:, j:j + n],
                                                    in1=v2[:, j:j + n], op=OP.max)
                            nc.scalar.activation(out=pb[:, j:j + n],
                                                 in_=vb[:, j:j + n], func=AF.Exp)
                            nc.gpsimd.memset(gbuf[:, k0:k0 + n, 128], 1.0)
                            for i in range(n):
                                ch = sc0 + k0 + i
                                ohp = gsb.tile([128, BLK], dt.float32, tag="ohp")
                                nc.vector.tensor_scalar(
                                    out=ohp[:], in0=iotas[:],
                                    scalar1=dlocs[:, ch:ch + 1],
                                    scalar2=pb[:, j + i:j + i + 1],
                                    op0=OP.is_equal, op1=OP.mult)
                                nc.tensor.matmul(
                                    out=acc_ps[:], lhsT=ohp[:],
                                    rhs=gbuf[:, k0 + i, 0:129],
                                    start=(done + i == 0),
                                    stop=(done + i == nch_b - 1))
                            done += n
                            j += n
                        # drain: out1 = relu(sum/s + b1); h2 = out1@W2 + c2
                        srec = gsb.tile([128, 1], dt.float32, tag="srec")
                        nc.vector.tensor_scalar_max(out=srec[:],
                                                    in0=acc_ps[:, 128:129],
                                                    scalar1=1e-30)
                        nc.vector.reciprocal(out=srec[:], in_=srec[:])
                        o1 = gsb.tile([128, 128], dt.float32, tag="o1")
                        nc.vector.tensor_scalar(out=o1[:], in0=acc_ps[:, 0:128],
                                                scalar1=srec[:], scalar2=None,
                                                op0=OP.mult)
                        nc.vector.tensor_tensor(out=o1[:], in0=o1[:],
                                                in1=W["b1b"][:], op=OP.add)
                        nc.vector.tensor_scalar_max(out=o1[:], in0=o1[:],
                                                    scalar1=0.0)
                        o1T_ps = ptrp.tile([128, 128], dt.float32, tag="o1T")
                        nc.tensor.transpose(out=o1T_ps[:], in_=o1[:],
                                            identity=idents[:])
                        o1T = gsb.tile([128, 128], dt.float32, tag="o1Ts")
                        nc.vector.tensor_copy(out=o1T[:], in_=o1T_ps[:])
                        h2_ps = pmmp.tile([128, 64], dt.float32, tag="h2")
                        nc.tensor.matmul(out=h2_ps[:], lhsT=o1T[:], rhs=W["W2"][:],
                                         start=True, stop=True)
                        h2 = gsb.tile([128, 64], dt.float32, tag="h2s")
                        nc.vector.tensor_tensor(out=h2[:], in0=h2_ps[:],
                                                in1=W["c2b"][:], op=OP.add)
                        nc.sync.dma_start(out=t2_sh[b * BLK:b * BLK + bn, :],
                                          in_=h2[:bn, :])

            nc.gpsimd.collective_compute(
                "AllGather", mybir.AluOpType.bypass, replica_groups=RG,
                ins=[t2_sh[0:H, :]], outs=[T2A[:]])
            nc.gpsimd.collective_compute(
                "AllGather", mybir.AluOpType.bypass, replica_groups=RG,
                ins=[t2_sh[H:SH, :]], outs=[T2B[:]])

            # ---------------- GCN passes ------------------------------------
            def gcn_pass(Tt, width, drain):
                with tc.tile_pool(name="gg", bufs=2) as gpool, \
                     tc.tile_pool(name="ggsb", bufs=2) as gsb2, \
                     tc.tile_pool(name="gacc", bufs=2, space="PSUM") as gacc, \
                     tc.tile_pool(name="gtr", bufs=2, space="PSUM") as gtr, \
                     tc.tile_pool(name="gmm", bufs=2, space="PSUM") as gmm:
                    for (sc0, nch, calls, blocks) in s.span_meta:
                        gbuf = gpool.tile([128, SMAX, 64], dt.float32, tag="ggb")
                        for (st, k0, n) in calls:
                            nc.gpsimd.dma_gather(
                                out_ap=gbuf[:, k0:k0 + n, :],
                                in_ap=st_ap(Tt, st),
                                idxs_ap=gidxs[:, (sc0 + k0) * 8:(sc0 + k0 + n) * 8],
                                num_idxs=n * CHUNK, num_idxs_reg=n * CHUNK,
                                elem_size=64)
                        for (b, runs) in blocks:
                            bn = LASTB if b == NBLK - 1 else BLK
                            acc_ps = gacc.tile([128, width], dt.float32, tag="acc")
                            nch_b = sum(n for (_, _, n) in runs)
                            done = 0
                            for (st, k0, n) in runs:
                                for i in range(n):
                                    ch = sc0 + k0 + i
                                    ohw = gsb2.tile([128, BLK], dt.float32,
                                                    tag="ohw")
                                    nc.vector.tensor_scalar(
                                        out=ohw[:], in0=iotas[:],
                                        scalar1=dlocs[:, ch:ch + 1],
                                        scalar2=wstrs[:, ch:ch + 1],
                                        op0=OP.is_equal, op1=OP.mult)
                                    nc.tensor.matmul(
                                        out=acc_ps[:], lhsT=ohw[:],
                                        rhs=gbuf[:, k0 + i, 0:width],
                                        start=(done + i == 0),
                                        stop=(done + i == nch_b - 1))
                                done += n
                            drain(b, bn, acc_ps, gsb2, gtr, gmm)

            def drain2(b, bn, acc_ps, gsb2, gtr, gmm):
                o2 = gsb2.tile([128, 64], dt.float32, tag="o2")
                nc.vector.tensor_tensor(out=o2[:], in0=acc_ps[:],
                                        in1=W["b2b"][:], op=OP.add)
                nc.vector.tensor_scalar_max(out=o2[:], in0=o2[:], scalar1=0.0)
                o2T_ps = gtr.tile([64, 128], dt.float32, tag="o2T")
                nc.tensor.transpose(out=o2T_ps[:], in_=o2[:], identity=idents[:])
                o2T = gsb2.tile([64, 128], dt.float32, tag="o2Ts")
                nc.vector.tensor_copy(out=o2T[:], in_=o2T_ps[:])
                h3_ps = gmm.tile([128, 32], dt.float32, tag="h3")
                nc.tensor.matmul(out=h3_ps[:], lhsT=o2T[:], rhs=W["W3"][:],
                                 start=True, stop=True)
                h3 = gsb2.tile([128, 64], dt.float32, tag="h3s")
                nc.vector.tensor_tensor(out=h3[:, 0:32], in0=h3_ps[:],
                                        in1=W["c3b"][:], op=OP.add)
                nc.vector.memset(h3[:, 32:64], 0.0)
                nc.sync.dma_start(out=t3_sh[b * BLK:b * BLK + bn, :],
                                  in_=h3[:bn, :])

            def drain3(b, bn, acc_ps, gsb2, gtr, gmm):
                o3 = gsb2.tile([128, 32], dt.float32, tag="o3")
                nc.vector.tensor_tensor(out=o3[:], in0=acc_ps[:, 0:32],
                                        in1=W["b3b"][:], op=OP.add)
                nc.vector.tensor_scalar_max(out=o3[:], in0=o3[:], scalar1=0.0)
                o3T_ps = gtr.tile([32, 128], dt.float32, tag="o3T")
                nc.tensor.transpose(out=o3T_ps[:], in_=o3[:], identity=idents[:])
                o3T = gsb2.tile([32, 128], dt.float32, tag="o3Ts")
                nc.vector.tensor_copy(out=o3T[:], in_=o3T_ps[:])
                zh_ps = gmm.tile([128, 128], dt.float32, tag="zh")
                nc.tensor.matmul(out=zh_ps[:], lhsT=o3T[:], rhs=W["zmzvw"][:],
                                 start=True, stop=True)
                zh = gsb2.tile([128, 128], dt.float32, tag="zhs")
                nc.vector.tensor_tensor(out=zh[:], in0=zh_ps[:],
                                        in1=W["zmzvb"][:], op=OP.add)
                zm = gsb2.tile([128, 64], dt.float32, tag="zm")
                nc.scalar.activation(out=zm[:], in_=zh[:, 0:64], func=AF.Sigmoid)
                eh = gsb2.tile([128, 64], dt.float32, tag="eh")
                nc.scalar.activation(out=eh[:], in_=zh[:, 64:128],
                                     func=AF.Exp, scale=0.5)
                epsb = gsb2.tile([128, 64], dt.float32, tag="epsb")
                nc.sync.dma_start(out=epsb[:bn, :],
                                  in_=eps_in[b * BLK:b * BLK + bn, :])
                zt = gsb2.tile([128, 64], dt.float32, tag="zt")
                nc.vector.tensor_tensor(out=zt[:bn, :], in0=eh[:bn, :],
                                        in1=epsb[:bn, :], op=OP.mult)
                nc.vector.tensor_tensor(out=zt[:bn, :], in0=zt[:bn, :],
                                        in1=zm[:bn, :], op=OP.add)
                nb0_ = b * BLK
                nc.sync.dma_start(out=zm_o[nb0_:nb0_ + bn, :], in_=zm[:bn, :])
                nc.sync.dma_start(out=zlv_o[nb0_:nb0_ + bn, :],
                                  in_=zh[:bn, 64:128])
                nc.sync.dma_start(out=z_o[nb0_:nb0_ + bn, :], in_=zt[:bn, :])

            gcn_pass((T2A, T2B), 64, drain2)
            nc.gpsimd.collective_compute(
                "AllGather", mybir.AluOpType.bypass, replica_groups=RG,
                ins=[t3_sh[0:H, :]], outs=[T3A[:]])
            nc.gpsimd.collective_compute(
                "AllGather", mybir.AluOpType.bypass, replica_groups=RG,
                ins=[t3_sh[H:SH, :]], outs=[T3B[:]])
            gcn_pass((T3A, T3B), 32, drain3)

    nc.compile()
    return nc


# ------------------------------------------------------------------ driver

_CACHE = {}


def _prepare(inputs):
    x = np.asarray(inputs["x"], f32)
    edge_src = np.asarray(inputs["edge_src"])
    edge_dst = np.asarray(inputs["edge_dst"])
    edge_w = np.asarray(inputs["edge_w"], f32)
    N = x.shape[0]
    s, gidx, dloc, wstr = _edge_schedule(edge_src, edge_dst, edge_w, N)
    w = _fold_weights({k: np.asarray(v, f32) for k, v in inputs.items()
                       if k not in ("x", "edge_src", "edge_dst", "edge_w", "eps")})
    eps = np.asarray(inputs["eps"], f32)
    iota = np.tile(np.arange(BLK, dtype=f32), (128, 1)).copy()
    ones_row = np.ones((1, 128), f32)
    in_maps = []
    SH = s.SH
    for c in range(NCORES):
        m = dict(w)
        m["xT"] = np.ascontiguousarray(x[c * SH:(c + 1) * SH].T)
        m["eps_s"] = np.ascontiguousarray(eps[c * SH:(c + 1) * SH])
        m["gidx"] = np.ascontiguousarray(gidx[c])
        m["dloc"] = np.ascontiguousarray(dloc[c])
        m["wstr"] = np.ascontiguousarray(wstr[c])
        m["iota"] = iota
        m["ones_row"] = ones_row
        in_maps.append(m)
    return N, s, in_maps


def get_runner(inputs):
    """Build (or fetch cached) program + runner; returns (run_fn, args)."""
    N, s, in_maps = _prepare(inputs)
    key = (N, s.SH, s.TOTC)
    if key not in _CACHE:
        nc = _build_program(N, s)
        from runner_inline import make_spmd_runner
        prep, run = make_spmd_runner(nc, NCORES)
        _CACHE[key] = (prep, run)
    prep, run = _CACHE[key]
    return run, prep(in_maps)


def kernel(**inputs):
    run, args = get_runner(inputs)
    res = run(args)
    zm = np.concatenate([res[c]["zm_o"] for c in range(NCORES)])
    zlv = np.concatenate([res[c]["zlv_o"] for c in range(NCORES)])
    z = np.concatenate([res[c]["z_o"] for c in range(NCORES)])
    return zm, zlv, z


# ---- inline runner module (kernel.py must be self-contained): create it ----
import os as _os
import sys as _sys
import types as _types

_RUNNER_SRC = '''
import numpy as np
import jax
from jax.sharding import Mesh, PartitionSpec
from jax.experimental.shard_map import shard_map
from concourse import mybir
from concourse.bass2jax import _bass_exec_p, install_neuronx_cc_hook, \\
    partition_id_tensor


def make_spmd_runner(nc, n_cores):
    install_neuronx_cc_hook()
    partition_name = nc.partition_id_tensor.name if nc.partition_id_tensor else None
    in_names, out_names, out_avals = [], [], []
    for alloc in nc.m.functions[0].allocations:
        if not isinstance(alloc, mybir.MemoryLocationSet):
            continue
        name = alloc.memorylocations[0].name
        if alloc.kind == "ExternalInput":
            if name != partition_name:
                in_names.append(name)
        elif alloc.kind == "ExternalOutput":
            out_names.append(name)
            out_avals.append(jax.core.ShapedArray(
                tuple(alloc.tensor_shape), mybir.dt.np(alloc.dtype)))
    n_params = len(in_names)
    all_in = in_names + out_names + ([partition_name] if partition_name else [])

    def _body(*args):
        operands = list(args)
        if partition_name is not None:
            operands.append(partition_id_tensor())
        outs = _bass_exec_p.bind(
            *operands, out_avals=tuple(out_avals), in_names=tuple(all_in),
            out_names=tuple(out_names), lowering_input_output_aliases=(),
            sim_require_finite=False, sim_require_nnan=False, nc=nc)
        return tuple(outs)

    devices = jax.devices()[:n_cores]
    mesh = Mesh(np.asarray(devices), ("core",))
    nio = n_params + len(out_names)
    fn = jax.jit(
        shard_map(_body, mesh=mesh, in_specs=(PartitionSpec("core"),) * nio,
                  out_specs=(PartitionSpec("core"),) * len(out_names),
                  check_rep=False),
        donate_argnums=tuple(range(n_params, nio)), keep_unused=True)

    def prep(in_maps):
        concat = [np.concatenate([np.asarray(in_maps[c][n])
                                  for c in range(n_cores)]) for n in in_names]
        return [jax.device_put(a) for a in concat]

    def run(args, block=True):
        zeros = [jax.device_put(
            np.zeros((n_cores * a.shape[0], *a.shape[1:]), a.dtype))
            for a in out_avals]
        outs = fn(*args, *zeros)
        if not block:
            return outs
        jax.block_until_ready(outs)
        return [{n: np.asarray(outs[i]).reshape(n_cores, *out_avals[i].shape)[c]
                 for i, n in enumerate(out_names)} for c in range(n_cores)]

    return prep, run
'''

if "runner_inline" not in _sys.modules:
    _mod = _types.ModuleType("runner_inline")
    exec(_RUNNER_SRC, _mod.__dict__)
    _sys.modules["runner_inline"] = _mod


# revision 7
# speedup vs baseline: 1.1747x; 1.1747x over previous
"""Trainium2 Bass kernel for nn_Encoder (GNN: BN+GAT+BN+GCN+BN+GCN+VAE head).

Self-contained: takes full inputs (as in reference.setup_inputs()), shards
destination nodes across 8 NeuronCores, runs one SPMD Bass program, returns
(z_mean, z_log_var, z).

Per core (dst-shard of N/8 nodes; its in-edges are a contiguous dst-sorted
range):
  Phase A: h1 = x @ W1fold + c1 (feature-major matmuls), att = h1 @ [a_i|a_j];
           write per-node 192-float rows [h1 | att_j | pad] to a table;
           AllGather the table.
  GAT: per 128-node dst block, dma_gather the 768B source rows of its edges
       (int16 indices -> 4 subtables of 25000 rows; edge chunks of 128 are
       (block, subtable)-pure, schedule padded to the max over cores so the
       SPMD program is identical on every core). One-hot(dst)*p matrices are
       built on DVE in one fused tensor_scalar; PE matmuls scatter-accumulate
       [sum p*h | sum p] into PSUM. att_i is broadcast per block via a rank-1
       PE matmul and reduced per edge with an ACT accumulate. Softmax uses no
       max-subtraction (score range is fp32-safe). Block drain computes out1
       and immediately h2 = out1@W2fold + c2 -> table 2 (so the next pass
       gathers 64-wide rows).
  GCN2/GCN3: same skeleton with one-hot(dst)*edge_w; GCN3's drain runs the
       VAE head inline. All BN layers are folded into weights on the host.
"""

import math
import numpy as np

NCORES = 8
BN_EPS = 1e-3
BLK = 128         # dst-block / one-hot window (nodes)
CHUNK = 128       # edges per scatter matmul (PE contraction dim)
NST = 4           # gather subtables (int16 index limit)
STSZ = 25000      # subtable rows
SPB = 2           # blocks per gather span
MAXC = 8          # max chunks (x128 idxs) per dma_gather call (desc-ring safe)

f32 = np.float32


# ----------------------------------------------------------------- host prep

def _fold_weights(i):
    s1 = (i["bn1_gamma"] / np.sqrt(i["bn1_var"] + BN_EPS)).astype(f32)
    t1 = (i["bn1_beta"] - i["bn1_mean"] * s1).astype(f32)
    W1 = (s1[:, None] * i["gat_w"]).astype(f32)            # [128,128]
    c1 = (t1 @ i["gat_w"]).astype(f32)                     # [128]
    s2 = (i["bn2_gamma"] / np.sqrt(i["bn2_var"] + BN_EPS)).astype(f32)
    t2 = (i["bn2_beta"] - i["bn2_mean"] * s2).astype(f32)
    W2 = (s2[:, None] * i["gcn2_w"]).astype(f32)           # [128,64]
    c2 = (t2 @ i["gcn2_w"]).astype(f32)                    # [64]
    s3 = (i["bn3_gamma"] / np.sqrt(i["bn3_var"] + BN_EPS)).astype(f32)
    t3 = (i["bn3_beta"] - i["bn3_mean"] * s3).astype(f32)
    W3 = (s3[:, None] * i["gcn3_w"]).astype(f32)           # [64,32]
    c3 = (t3 @ i["gcn3_w"]).astype(f32)                    # [32]
    B = BLK
    return dict(
        W1=W1, c1=c1[:, None].copy(),                       # c1 as [128,1]
        aa=np.stack([i["gat_a_self"], i["gat_a_nbr"]], 1).astype(f32),  # [128,2]
        W2=W2, c2b=np.tile(c2, (B, 1)),                     # [128,64]
        W3=W3, c3b=np.tile(c3, (B, 1)),                     # [128,32]
        b1b=np.tile(i["gat_b"].astype(f32), (B, 1)),        # [128,128]
        b2b=np.tile(i["gcn2_b"].astype(f32), (B, 1)),       # [128,64]
        b3b=np.tile(i["gcn3_b"].astype(f32), (B, 1)),       # [128,32]
        zmzvw=np.concatenate([i["zm_w"], i["zv_w"]], 1).astype(f32),  # [32,128]
        zmzvb=np.tile(np.concatenate([i["zm_b"], i["zv_b"]]).astype(f32),
                      (B, 1)),                              # [128,128]
    )


class Sched:
    pass


def _edge_schedule(edge_src, edge_dst, edge_w, N):
    """Static SPMD schedule. Chunk order: span -> subtable -> block.
    Per-(block,subtable) chunk counts are the max over cores."""
    SH = N // NCORES
    NBLK = math.ceil(SH / BLK)

    def st_loc(src):
        st = src // STSZ
        return st, src - st * STSZ

    bounds = np.searchsorted(edge_dst, np.arange(NCORES + 1) * SH)
    cnts = np.zeros((NCORES, NBLK, NST), np.int64)
    for c in range(NCORES):
        e = slice(bounds[c], bounds[c + 1])
        b = (edge_dst[e] - c * SH) // BLK
        st, _ = st_loc(edge_src[e])
        np.add.at(cnts[c], (b, st), 1)
    CPBS = -(-cnts.max(0) // CHUNK)                        # [NBLK, NST]
    CPBS[:, 0] = np.maximum(1, CPBS[:, 0])                 # every block runs
    spans = [(b0, min(b0 + SPB, NBLK)) for b0 in range(0, NBLK, SPB)]

    # global chunk bases in span->st->block order
    c0 = np.zeros((NBLK, NST), np.int64)
    acc = 0
    for (b0, b1) in spans:
        for st in range(NST):
            for b in range(b0, b1):
                c0[b, st] = acc
                acc += CPBS[b, st]
    TOTC = acc

    # per-span metadata
    span_meta = []
    for (b0, b1) in spans:
        sc0 = int(c0[b0, 0])                               # first chunk of span
        nch = int(CPBS[b0:b1].sum())
        calls = []
        for st in range(NST):
            r0 = int(c0[b0, st])
            rn = int(CPBS[b0:b1, st].sum())
            while rn > 0:
                n = min(rn, MAXC)
                calls.append((st, r0 - sc0, n))            # (subtable, slot0, nch)
                r0 += n
                rn -= n
        blocks = []
        for b in range(b0, b1):
            runs = [(st, int(c0[b, st]) - sc0, int(CPBS[b, st]))
                    for st in range(NST) if CPBS[b, st] > 0]
            blocks.append((b, runs))
        span_meta.append((sc0, nch, calls, blocks))
    SMAX = max(m[1] for m in span_meta)
    CPBMAX = int(CPBS.sum(1).max())

    # streams
    gidx = np.zeros((NCORES, 16, TOTC * 8), np.int16)
    dloc = np.full((NCORES, 128, TOTC), -1.0, f32)
    wstr = np.zeros((NCORES, 128, TOTC), f32)
    for c in range(NCORES):
        e = slice(bounds[c], bounds[c + 1])
        src = edge_src[e]
        d = edge_dst[e] - c * SH
        w = edge_w[e]
        b = d // BLK
        st, loc = st_loc(src)
        key = b * NST + st
        order = np.argsort(key, kind="stable")
        src, d, w, b, st, key, loc = (a[order]
                                      for a in (src, d, w, b, st, key, loc))
        kcnt = np.bincount(key, minlength=NBLK * NST)
        kstart = np.concatenate([[0], np.cumsum(kcnt)])[:-1]
        pos = np.arange(len(src)) - kstart[key]
        col = c0[b, st] + pos // CHUNK
        row = pos % CHUNK
        gidx[c, row % 16, col * 8 + row // 16] = loc.astype(np.int16)
        dloc[c, row, col] = (d - b * BLK).astype(f32)
        wstr[c, row, col] = w

    s = Sched()
    s.SH, s.NBLK, s.TOTC, s.SMAX, s.CPBMAX = SH, NBLK, TOTC, SMAX, CPBMAX
    s.CPBS, s.c0, s.span_meta = CPBS, c0, span_meta
    s.LASTB = SH - BLK * (NBLK - 1)
    s.stdef = [(st * STSZ, min(STSZ, max(0, N - st * STSZ)))
               for st in range(NST)]
    return s, np.tile(gidx, (1, 8, 1)), dloc, wstr


# ------------------------------------------------------------- device program

def _build_program(N, s):
    import concourse.tile as tile
    from concourse import bacc, mybir
    from concourse.masks import make_identity
    import contextlib

    dt = mybir.dt
    AF = mybir.ActivationFunctionType
    OP = mybir.AluOpType
    SH, NBLK, TOTC, SMAX = s.SH, s.NBLK, s.TOTC, s.SMAX
    LASTB = s.LASTB

    nc = bacc.Bacc("TRN2", num_devices=NCORES, target_bir_lowering=False,
                   dynamic_dma_scratch_size=16384)

    xT_in = nc.dram_tensor("xT", [128, SH], dt.float32, kind="ExternalInput")
    eps_in = nc.dram_tensor("eps_s", [SH, 64], dt.float32, kind="ExternalInput")
    gidx_in = nc.dram_tensor("gidx", [128, TOTC * 8], dt.int16, kind="ExternalInput")
    dloc_in = nc.dram_tensor("dloc", [128, TOTC], dt.float32, kind="ExternalInput")
    wstr_in = nc.dram_tensor("wstr", [128, TOTC], dt.float32, kind="ExternalInput")
    wshapes = dict(W1=[128, 128], c1=[128, 1], aa=[128, 2], W2=[128, 64],
                   c2b=[BLK, 64], W3=[64, 32], c3b=[BLK, 32], b1b=[BLK, 128],
                   b2b=[BLK, 64], b3b=[BLK, 32], zmzvw=[32, 128],
                   zmzvb=[BLK, 128])
    win = {n: nc.dram_tensor(n, sh, dt.float32, kind="ExternalInput")
           for n, sh in wshapes.items()}
    iota_in = nc.dram_tensor("iota", [128, BLK], dt.float32, kind="ExternalInput")
    ones_in = nc.dram_tensor("ones_row", [1, 128], dt.float32, kind="ExternalInput")

    zm_o = nc.dram_tensor("zm_o", [SH, 64], dt.float32, kind="ExternalOutput")
    zlv_o = nc.dram_tensor("zlv_o", [SH, 64], dt.float32, kind="ExternalOutput")
    z_o = nc.dram_tensor("z_o", [SH, 64], dt.float32, kind="ExternalOutput")

    aug_sh = nc.dram_tensor("aug_sh", [SH, 192], dt.float32)
    T1 = nc.dram_tensor("T1", [N, 192], dt.float32)
    t2_sh = nc.dram_tensor("t2_sh", [SH, 64], dt.float32)
    T2 = nc.dram_tensor("T2", [N, 64], dt.float32)
    t3_sh = nc.dram_tensor("t3_sh", [SH, 64], dt.float32)
    T3 = nc.dram_tensor("T3", [N, 64], dt.float32)
    atti_d = nc.dram_tensor("atti_d", [1, SH], dt.float32)

    RG = [list(range(NCORES))]

    def st_ap(tab, st):
        base, size = s.stdef[st]
        return tab[base:base + size, :]

    with tile.TileContext(nc, num_cores=NCORES) as tc:
        with contextlib.ExitStack() as ctx:
            from concourse import library_config
            nc.gpsimd.load_library(library_config.mlp)
            pers = ctx.enter_context(tc.tile_pool(name="pers", bufs=1))
            W = {}
            for n, sh in wshapes.items():
                W[n] = pers.tile(sh, dt.float32, name=f"w_{n}")
                nc.sync.dma_start(out=W[n][:], in_=win[n][:])
            iotas = pers.tile([128, BLK], dt.float32)
            nc.sync.dma_start(out=iotas[:], in_=iota_in[:])
            oness = pers.tile([1, 128], dt.float32)
            nc.sync.dma_start(out=oness[:], in_=ones_in[:])
            idents = pers.tile([128, 128], dt.float32)
            make_identity(nc, idents[:])
            gidxs = pers.tile([128, TOTC * 8], dt.int16)
            nc.sync.dma_start(out=gidxs[:], in_=gidx_in[:])
            dlocs = pers.tile([128, TOTC], dt.float32)
            nc.sync.dma_start(out=dlocs[:], in_=dloc_in[:])
            wstrs = pers.tile([128, TOTC], dt.float32)
            nc.sync.dma_start(out=wstrs[:], in_=wstr_in[:])

            # ---------------- phase A ---------------------------------------
            with tc.tile_pool(name="pa_sb", bufs=3) as pa, \
                 tc.tile_pool(name="pa_xt", bufs=1) as paxt, \
                 tc.tile_pool(name="pa_ps", bufs=2, space="PSUM") as pap, \
                 tc.tile_pool(name="pa_ps2", bufs=2, space="PSUM") as pap2:
                xTs = paxt.tile([128, SH], dt.float32)
                nc.sync.dma_start(out=xTs[:], in_=xT_in[:])
                for sb_i in range(NBLK):
                    n0 = sb_i * BLK
                    ns = min(BLK, SH - n0)
                    h1T_ps = pap.tile([128, BLK], dt.float32, tag="h1T")
                    nc.tensor.matmul(out=h1T_ps[:, :ns], lhsT=W["W1"][:],
                                     rhs=xTs[:, n0:n0 + ns], start=True, stop=True)
                    h1T_sb = pa.tile([128, BLK], dt.float32, tag="h1Ts")
                    nc.scalar.activation(out=h1T_sb[:, :ns], in_=h1T_ps[:, :ns],
                                         func=AF.Identity, bias=W["c1"][:])
                    att_ps = pap2.tile([2, BLK], dt.float32, tag="att")
                    nc.tensor.matmul(out=att_ps[:, :ns], lhsT=W["aa"][:],
                                     rhs=h1T_sb[:, :ns], start=True, stop=True)
                    att_sb = pa.tile([2, BLK], dt.float32, tag="att_sb")
                    nc.vector.tensor_copy(out=att_sb[:, :ns], in_=att_ps[:, :ns])
                    nc.sync.dma_start(out=atti_d[0:1, n0:n0 + ns],
                                      in_=att_sb[0:1, :ns])
                    tr_ps = pap.tile([128, BLK], dt.float32, tag="tr")
                    nc.tensor.transpose(out=tr_ps[:ns, :], in_=h1T_sb[:, :ns],
                                        identity=idents[:])
                    attT_ps = pap2.tile([128, 2], dt.float32, tag="attT")
                    nc.tensor.transpose(out=attT_ps[:ns, :], in_=att_sb[:, :ns],
                                        identity=idents[0:2, 0:2])
                    aug_sb = pa.tile([128, 192], dt.float32, tag="aug")
                    nc.vector.tensor_copy(out=aug_sb[:ns, 0:128], in_=tr_ps[:ns, :])
                    nc.vector.tensor_copy(out=aug_sb[:ns, 128:129],
                                          in_=attT_ps[:ns, 1:2])
                    nc.vector.memset(aug_sb[:ns, 129:192], 0.0)
                    nc.sync.dma_start(out=aug_sh[n0:n0 + ns, :], in_=aug_sb[:ns, :])

            nc.gpsimd.collective_compute(
                "AllGather", mybir.AluOpType.bypass, replica_groups=RG,
                ins=[aug_sh[:]], outs=[T1[:]])

            # ---------------- GAT pass --------------------------------------
            with tc.tile_pool(name="g1", bufs=2) as g1pool, \
                 tc.tile_pool(name="gsb", bufs=2) as gsb, \
                 tc.tile_pool(name="gsc", bufs=2) as gsc, \
                 tc.tile_pool(name="pacc", bufs=2, space="PSUM") as pacc, \
                 tc.tile_pool(name="pab", bufs=2, space="PSUM") as pabp, \
                 tc.tile_pool(name="ptr", bufs=2, space="PSUM") as ptrp, \
                 tc.tile_pool(name="pmm", bufs=2, space="PSUM") as pmmp:
                for (sc0, nch, calls, blocks) in s.span_meta:
                    gbuf = g1pool.tile([128, SMAX, 192], dt.float32, tag="g1b")
                    for (st, k0, n) in calls:
                        nc.gpsimd.dma_gather(
                            out_ap=gbuf[:, k0:k0 + n, :],
                            in_ap=st_ap(T1, st),
                            idxs_ap=gidxs[:, (sc0 + k0) * 8:(sc0 + k0 + n) * 8],
                            num_idxs=n * CHUNK, num_idxs_reg=n * CHUNK,
                            elem_size=192)
                    for (b, runs) in blocks:
                        bn = LASTB if b == NBLK - 1 else BLK
                        # att_i broadcast for this block
                        atti_sb = gsb.tile([1, BLK], dt.float32, tag="atti")
                        nc.sync.dma_start(out=atti_sb[0:1, :bn],
                                          in_=atti_d[0:1, b * BLK:b * BLK + bn])
                        if bn < BLK:
                            nc.vector.memset(atti_sb[0:1, bn:BLK], 0.0)
                        ab_ps = pabp.tile([128, BLK], dt.float32, tag="ab")
                        nc.tensor.matmul(out=ab_ps[:], lhsT=oness[:],
                                         rhs=atti_sb[:], start=True, stop=True)
                        ab_sb = gsb.tile([128, BLK], dt.float32, tag="ab_sb")
                        nc.vector.tensor_copy(out=ab_sb[:], in_=ab_ps[:])
                        attie = gsc.tile([128, s.CPBMAX], dt.float32, tag="attie")
                        vb = gsc.tile([128, s.CPBMAX], dt.float32, tag="vb")
                        v2 = gsc.tile([128, s.CPBMAX], dt.float32, tag="v2")
                        pb = gsc.tile([128, s.CPBMAX], dt.float32, tag="pb")
                        ohsc = gsc.tile([128, BLK], dt.float32, tag="ohsc")
                        acc_ps = pacc.tile([128, 129], dt.float32, tag="acc")
                        nch_b = sum(n for (_, _, n) in runs)
                        done = 0
                        j = 0
                        for (st, k0, n) in runs:
                            for i in range(n):
                                ch = sc0 + k0 + i
                                oh = gsb.tile([128, BLK], dt.float32, tag="oh", bufs=4)
                                nc.vector.tensor_scalar(
                                    out=oh[:], in0=iotas[:],
                                    scalar1=dlocs[:, ch:ch + 1], scalar2=None,
                                    op0=OP.is_equal)
                                ohA = gsb.tile([128, BLK], dt.float32, tag="ohA", bufs=4)
                                nc.gpsimd.tensor_tensor(out=ohA[:], in0=oh[:],
                                                        in1=ab_sb[:], op=OP.mult)
                                nc.scalar.activation(out=ohsc[:], in_=ohA[:],
                                                     func=AF.Identity,
                                                     accum_out=attie[:, j + i:j + i + 1])
                            # batched v = attj + attie ; p = exp(max(v, .2v))
                            nc.vector.tensor_tensor(
                                out=vb[:, j:j + n], in0=gbuf[:, k0:k0 + n, 128],
                                in1=attie[:, j:j + n], op=OP.add)
                            nc.vector.tensor_scalar_mul(out=v2[:, j:j + n],
                                                        in0=vb[:, j:j + n],
                                                        scalar1=0.2)
                            nc.vector.tensor_tensor(out=vb[:, j:j + n],
                                                    in0=vb[:, j:j + n],
                                                    in1=v2[:, j:j + n], op=OP.max)
                            nc.scalar.activation(out=pb[:, j:j + n],
                                                 in_=vb[:, j:j + n], func=AF.Exp)
                            nc.gpsimd.memset(gbuf[:, k0:k0 + n, 128], 1.0)
                            for i in range(n):
                                ch = sc0 + k0 + i
                                ohp = gsb.tile([128, BLK], dt.float32, tag="ohp", bufs=4)
                                nc.vector.tensor_scalar(
                                    out=ohp[:], in0=iotas[:],
                                    scalar1=dlocs[:, ch:ch + 1],
                                    scalar2=pb[:, j + i:j + i + 1],
                                    op0=OP.is_equal, op1=OP.mult)
                                nc.tensor.matmul(
                                    out=acc_ps[:], lhsT=ohp[:],
                                    rhs=gbuf[:, k0 + i, 0:129],
                                    start=(done + i == 0),
                                    stop=(done + i == nch_b - 1))
                            done += n
                            j += n
                        # drain: out1 = relu(sum/s + b1); h2 = out1@W2 + c2
                        srec = gsb.tile([128, 1], dt.float32, tag="srec")
                        nc.vector.tensor_scalar_max(out=srec[:],
                                                    in0=acc_ps[:, 128:129],
                                                    scalar1=1e-30)
                        nc.vector.reciprocal(out=srec[:], in_=srec[:])
                        o1 = gsb.tile([128, 128], dt.float32, tag="o1")
                        nc.vector.tensor_scalar(out=o1[:], in0=acc_ps[:, 0:128],
                                                scalar1=srec[:], scalar2=None,
                                                op0=OP.mult)
                        nc.vector.tensor_tensor(out=o1[:], in0=o1[:],
                                                in1=W["b1b"][:], op=OP.add)
                        nc.vector.tensor_scalar_max(out=o1[:], in0=o1[:],
                                                    scalar1=0.0)
                        o1T_ps = ptrp.tile([128, 128], dt.float32, tag="o1T")
                        nc.tensor.transpose(out=o1T_ps[:], in_=o1[:],
                                            identity=idents[:])
                        o1T = gsb.tile([128, 128], dt.float32, tag="o1Ts")
                        nc.vector.tensor_copy(out=o1T[:], in_=o1T_ps[:])
                        h2_ps = pmmp.tile([128, 64], dt.float32, tag="h2")
                        nc.tensor.matmul(out=h2_ps[:], lhsT=o1T[:], rhs=W["W2"][:],
                                         start=True, stop=True)
                        h2 = gsb.tile([128, 64], dt.float32, tag="h2s")
                        nc.vector.tensor_tensor(out=h2[:], in0=h2_ps[:],
                                                in1=W["c2b"][:], op=OP.add)
                        nc.sync.dma_start(out=t2_sh[b * BLK:b * BLK + bn, :],
                                          in_=h2[:bn, :])

            nc.gpsimd.collective_compute(
                "AllGather", mybir.AluOpType.bypass, replica_groups=RG,
                ins=[t2_sh[:]], outs=[T2[:]])

            # ---------------- GCN passes ------------------------------------
            def gcn_pass(Tt, width, drain):
                with tc.tile_pool(name="gg", bufs=2) as gpool, \
                     tc.tile_pool(name="ggsb", bufs=2) as gsb2, \
                     tc.tile_pool(name="gacc", bufs=2, space="PSUM") as gacc, \
                     tc.tile_pool(name="gtr", bufs=2, space="PSUM") as gtr, \
                     tc.tile_pool(name="gmm", bufs=2, space="PSUM") as gmm:
                    for (sc0, nch, calls, blocks) in s.span_meta:
                        gbuf = gpool.tile([128, SMAX, 64], dt.float32, tag="ggb")
                        for (st, k0, n) in calls:
                            nc.gpsimd.dma_gather(
                                out_ap=gbuf[:, k0:k0 + n, :],
                                in_ap=st_ap(Tt, st),
                                idxs_ap=gidxs[:, (sc0 + k0) * 8:(sc0 + k0 + n) * 8],
                                num_idxs=n * CHUNK, num_idxs_reg=n * CHUNK,
                                elem_size=64)
                        for (b, runs) in blocks:
                            bn = LASTB if b == NBLK - 1 else BLK
                            acc_ps = gacc.tile([128, width], dt.float32, tag="acc")
                            nch_b = sum(n for (_, _, n) in runs)
                            done = 0
                            for (st, k0, n) in runs:
                                for i in range(n):
                                    ch = sc0 + k0 + i
                                    ohw = gsb2.tile([128, BLK], dt.float32,
                                                    tag="ohw", bufs=4)
                                    nc.vector.tensor_scalar(
                                        out=ohw[:], in0=iotas[:],
                                        scalar1=dlocs[:, ch:ch + 1],
                                        scalar2=wstrs[:, ch:ch + 1],
                                        op0=OP.is_equal, op1=OP.mult)
                                    nc.tensor.matmul(
                                        out=acc_ps[:], lhsT=ohw[:],
                                        rhs=gbuf[:, k0 + i, 0:width],
                                        start=(done + i == 0),
                                        stop=(done + i == nch_b - 1))
                                done += n
                            drain(b, bn, acc_ps, gsb2, gtr, gmm)

            def drain2(b, bn, acc_ps, gsb2, gtr, gmm):
                o2 = gsb2.tile([128, 64], dt.float32, tag="o2")
                nc.vector.tensor_tensor(out=o2[:], in0=acc_ps[:],
                                        in1=W["b2b"][:], op=OP.add)
                nc.vector.tensor_scalar_max(out=o2[:], in0=o2[:], scalar1=0.0)
                o2T_ps = gtr.tile([64, 128], dt.float32, tag="o2T")
                nc.tensor.transpose(out=o2T_ps[:], in_=o2[:], identity=idents[:])
                o2T = gsb2.tile([64, 128], dt.float32, tag="o2Ts")
                nc.vector.tensor_copy(out=o2T[:], in_=o2T_ps[:])
                h3_ps = gmm.tile([128, 32], dt.float32, tag="h3")
                nc.tensor.matmul(out=h3_ps[:], lhsT=o2T[:], rhs=W["W3"][:],
                                 start=True, stop=True)
                h3 = gsb2.tile([128, 64], dt.float32, tag="h3s")
                nc.vector.tensor_tensor(out=h3[:, 0:32], in0=h3_ps[:],
                                        in1=W["c3b"][:], op=OP.add)
                nc.vector.memset(h3[:, 32:64], 0.0)
                nc.sync.dma_start(out=t3_sh[b * BLK:b * BLK + bn, :],
                                  in_=h3[:bn, :])

            def drain3(b, bn, acc_ps, gsb2, gtr, gmm):
                o3 = gsb2.tile([128, 32], dt.float32, tag="o3")
                nc.vector.tensor_tensor(out=o3[:], in0=acc_ps[:, 0:32],
                                        in1=W["b3b"][:], op=OP.add)
                nc.vector.tensor_scalar_max(out=o3[:], in0=o3[:], scalar1=0.0)
                o3T_ps = gtr.tile([32, 128], dt.float32, tag="o3T")
                nc.tensor.transpose(out=o3T_ps[:], in_=o3[:], identity=idents[:])
                o3T = gsb2.tile([32, 128], dt.float32, tag="o3Ts")
                nc.vector.tensor_copy(out=o3T[:], in_=o3T_ps[:])
                zh_ps = gmm.tile([128, 128], dt.float32, tag="zh")
                nc.tensor.matmul(out=zh_ps[:], lhsT=o3T[:], rhs=W["zmzvw"][:],
                                 start=True, stop=True)
                zh = gsb2.tile([128, 128], dt.float32, tag="zhs")
                nc.vector.tensor_tensor(out=zh[:], in0=zh_ps[:],
                                        in1=W["zmzvb"][:], op=OP.add)
                zm = gsb2.tile([128, 64], dt.float32, tag="zm")
                nc.scalar.activation(out=zm[:], in_=zh[:, 0:64], func=AF.Sigmoid)
                eh = gsb2.tile([128, 64], dt.float32, tag="eh")
                nc.scalar.activation(out=eh[:], in_=zh[:, 64:128],
                                     func=AF.Exp, scale=0.5)
                epsb = gsb2.tile([128, 64], dt.float32, tag="epsb")
                nc.sync.dma_start(out=epsb[:bn, :],
                                  in_=eps_in[b * BLK:b * BLK + bn, :])
                zt = gsb2.tile([128, 64], dt.float32, tag="zt")
                nc.vector.tensor_tensor(out=zt[:bn, :], in0=eh[:bn, :],
                                        in1=epsb[:bn, :], op=OP.mult)
                nc.vector.tensor_tensor(out=zt[:bn, :], in0=zt[:bn, :],
                                        in1=zm[:bn, :], op=OP.add)
                nb0_ = b * BLK
                nc.sync.dma_start(out=zm_o[nb0_:nb0_ + bn, :], in_=zm[:bn, :])
                nc.sync.dma_start(out=zlv_o[nb0_:nb0_ + bn, :],
                                  in_=zh[:bn, 64:128])
                nc.sync.dma_start(out=z_o[nb0_:nb0_ + bn, :], in_=zt[:bn, :])

            gcn_pass(T2, 64, drain2)
            nc.gpsimd.collective_compute(
                "AllGather", mybir.AluOpType.bypass, replica_groups=RG,
                ins=[t3_sh[:]], outs=[T3[:]])
            gcn_pass(T3, 32, drain3)

    nc.compile()
    return nc


# ------------------------------------------------------------------ driver

_CACHE = {}


def _prepare(inputs):
    x = np.asarray(inputs["x"], f32)
    edge_src = np.asarray(inputs["edge_src"])
    edge_dst = np.asarray(inputs["edge_dst"])
    edge_w = np.asarray(inputs["edge_w"], f32)
    N = x.shape[0]
    s, gidx, dloc, wstr = _edge_schedule(edge_src, edge_dst, edge_w, N)
    w = _fold_weights({k: np.asarray(v, f32) for k, v in inputs.items()
                       if k not in ("x", "edge_src", "edge_dst", "edge_w", "eps")})
    eps = np.asarray(inputs["eps"], f32)
    iota = np.tile(np.arange(BLK, dtype=f32), (128, 1)).copy()
    ones_row = np.ones((1, 128), f32)
    in_maps = []
    SH = s.SH
    for c in range(NCORES):
        m = dict(w)
        m["xT"] = np.ascontiguousarray(x[c * SH:(c + 1) * SH].T)
        m["eps_s"] = np.ascontiguousarray(eps[c * SH:(c + 1) * SH])
        m["gidx"] = np.ascontiguousarray(gidx[c])
        m["dloc"] = np.ascontiguousarray(dloc[c])
        m["wstr"] = np.ascontiguousarray(wstr[c])
        m["iota"] = iota
        m["ones_row"] = ones_row
        in_maps.append(m)
    return N, s, in_maps


def get_runner(inputs):
    """Build (or fetch cached) program + runner; returns (run_fn, args)."""
    N, s, in_maps = _prepare(inputs)
    key = (N, s.SH, s.TOTC)
    if key not in _CACHE:
        nc = _build_program(N, s)
        from runner_inline import make_spmd_runner
        prep, run = make_spmd_runner(nc, NCORES)
        _CACHE[key] = (prep, run)
    prep, run = _CACHE[key]
    return run, prep(in_maps)


def kernel(**inputs):
    run, args = get_runner(inputs)
    res = run(args)
    zm = np.concatenate([res[c]["zm_o"] for c in range(NCORES)])
    zlv = np.concatenate([res[c]["zlv_o"] for c in range(NCORES)])
    z = np.concatenate([res[c]["z_o"] for c in range(NCORES)])
    return zm, zlv, z


# ---- inline runner module (kernel.py must be self-contained): create it ----
import os as _os
import sys as _sys
import types as _types

_RUNNER_SRC = '''
import numpy as np
import jax
from jax.sharding import Mesh, PartitionSpec
from jax.experimental.shard_map import shard_map
from concourse import mybir
from concourse.bass2jax import _bass_exec_p, install_neuronx_cc_hook, \\
    partition_id_tensor


def make_spmd_runner(nc, n_cores):
    install_neuronx_cc_hook()
    partition_name = nc.partition_id_tensor.name if nc.partition_id_tensor else None
    in_names, out_names, out_avals = [], [], []
    for alloc in nc.m.functions[0].allocations:
        if not isinstance(alloc, mybir.MemoryLocationSet):
            continue
        name = alloc.memorylocations[0].name
        if alloc.kind == "ExternalInput":
            if name != partition_name:
                in_names.append(name)
        elif alloc.kind == "ExternalOutput":
            out_names.append(name)
            out_avals.append(jax.core.ShapedArray(
                tuple(alloc.tensor_shape), mybir.dt.np(alloc.dtype)))
    n_params = len(in_names)
    all_in = in_names + out_names + ([partition_name] if partition_name else [])

    def _body(*args):
        operands = list(args)
        if partition_name is not None:
            operands.append(partition_id_tensor())
        outs = _bass_exec_p.bind(
            *operands, out_avals=tuple(out_avals), in_names=tuple(all_in),
            out_names=tuple(out_names), lowering_input_output_aliases=(),
            sim_require_finite=False, sim_require_nnan=False, nc=nc)
        return tuple(outs)

    devices = jax.devices()[:n_cores]
    mesh = Mesh(np.asarray(devices), ("core",))
    nio = n_params + len(out_names)
    fn = jax.jit(
        shard_map(_body, mesh=mesh, in_specs=(PartitionSpec("core"),) * nio,
                  out_specs=(PartitionSpec("core"),) * len(out_names),
                  check_rep=False),
        donate_argnums=tuple(range(n_params, nio)), keep_unused=True)

    def prep(in_maps):
        concat = [np.concatenate([np.asarray(in_maps[c][n])
                                  for c in range(n_cores)]) for n in in_names]
        return [jax.device_put(a) for a in concat]

    def run(args, block=True):
        zeros = [jax.device_put(
            np.zeros((n_cores * a.shape[0], *a.shape[1:]), a.dtype))
            for a in out_avals]
        outs = fn(*args, *zeros)
        if not block:
            return outs
        jax.block_until_ready(outs)
        return [{n: np.asarray(outs[i]).reshape(n_cores, *out_avals[i].shape)[c]
                 for i, n in enumerate(out_names)} for c in range(n_cores)]

    return prep, run
'''

if "runner_inline" not in _sys.modules:
    _mod = _types.ModuleType("runner_inline")
    exec(_RUNNER_SRC, _mod.__dict__)
    _sys.modules["runner_inline"] = _mod
